# revision 6
# baseline (speedup 1.0000x reference)
"""Trainium2 Bass kernel for windowed Conv1d(k=3) + sigmoid gating.

Reference (B=16, T=960, D=1024, W=10): windows of 10 conv'd independently
with per-window zero pad 1:
    cnn[t, d] = sum_{k,c} conv_w[d, c, k] * xpad[t + k, c]
    out = cnn * sigmoid(cnn @ gate_w.T + gate_b)

Strategy: data parallel over 8 cores (2 batches / 192 windows / core).
The conv uses hybrid Winograd: two F(4,3) tiles (outputs 0-3 from
xp[0:6], outputs 4-7 from xp[4:10]) at points {0,1,-1,2,-1/2,inf} plus an
F(2,3) tail (outputs 8,9 from xp[8:12]) whose 4 products FOLD into the
same 6 weight matrices by choosing the tail's interpolation points as a
subset of the F43 points.  Rotating the tail point-set over 3 window
classes (w%3) equalizes the streams:
    T1=(0,1,-1,inf)->streams(0,1,2,5)  T2=(0,2,-1/2,inf)->(0,3,4,5)
    T3=(1,-1,2,-1/2)->(1,2,3,4)
so every stream carries exactly 8 columns per 3 windows: 16 muls/window
(vs 30 direct, 20 for F(2,3)) => conv PE time ~82us + gate ~52us.

Streams use canonical Vandermonde weights g_b = [1,b,b^2] . W (the
normalization lives in the host-side input transforms V^{-T}).  Per core:
2 groups of 96 windows; per (group, dck): 6 accumulation chains of N=256
(PSUM: 3 banks/slot, 2 slots + 2 gate banks = 8).  The A^T combine runs
on ScalarE/VectorE/GpSimd under the matmul stream with bf16 intermediates.
Host does padding, transposition to channel-major, input transforms, and
weight transforms (f64 -> bf16).
"""

import numpy as np
import ml_dtypes

import concourse.bacc as bacc
import concourse.bass as bass
import concourse.tile as tile
from concourse import mybir
from concourse.bass_utils import run_bass_kernel_spmd

BF16 = ml_dtypes.bfloat16

B, T, D, W = 16, 960, 1024, 10
NCORES = 8
BC = B // NCORES             # batches per core (2)
NWIN = BC * T // W           # windows per core (192)
RC = NWIN * W                # output rows per core (1920)
NG = 2                       # groups per core
GWN = NWIN // NG             # windows per group (96)
GN = GWN * W                 # output cols per group (960)
NS = 6                       # winograd streams
SCOL = GWN * 16 // 6         # stream cols per group (256)
NCH = D // 128               # d chunks (8)
NT = GWN // 3                # tail windows per class per group (32)
AF = mybir.ActivationFunctionType

INF = "inf"
PTS = [0.0, 1.0, -1.0, 2.0, -0.5, INF]
T1_PTS = [0.0, 1.0, -1.0, INF]; T1_MAP = [0, 1, 2, 5]
T2_PTS = [0.0, 2.0, -0.5, INF]; T2_MAP = [0, 3, 4, 5]
T3_PTS = [1.0, -1.0, 2.0, -0.5]; T3_MAP = [1, 2, 3, 4]
# class feeding stream j at slot X / Y (class index 0/1/2 = T1/T2/T3)
XY = {0: (0, 1), 1: (0, 2), 2: (0, 2), 3: (1, 2), 4: (1, 2), 5: (0, 1)}
TAIL = [(T1_PTS, T1_MAP), (T2_PTS, T2_MAP), (T3_PTS, T3_MAP)]


def _vinv_T(points):
    n = len(points)
    V = np.zeros((n, n))
    for j, b in enumerate(points):
        if b is INF:
            V[j, n - 1] = 1.0
        else:
            V[j] = [b ** i for i in range(n)]
    return np.linalg.inv(V).T


BA = _vinv_T(PTS)
BT = [_vinv_T(p) for p, _ in TAIL]


def _build():
    nc = bacc.Bacc("TRN2", target_bir_lowering=False, debug=False)

    # xt[g*6+j]: [cc, (ck, col)] transformed input, one DMA per (g, j)
    xt = nc.dram_tensor("xt", [NG * NS, 128, NCH * SCOL], mybir.dt.bfloat16,
                        kind="ExternalInput")
    # cwr[dck]: [cc, ((j*NCH+ck)*128 + dd)] conv lhsT blocks
    cwr = nc.dram_tensor("cwr", [NCH, 128, NS * NCH * 128], mybir.dt.bfloat16,
                         kind="ExternalInput")
    # gwr[eck]: [dd, (dck*128 + ee)] gate lhsT blocks
    gwr = nc.dram_tensor("gwr", [NCH, 128, NCH * 128], mybir.dt.bfloat16,
                         kind="ExternalInput")
    cb = nc.dram_tensor("cb", [128, NCH], mybir.dt.float32, kind="ExternalInput")
    gb = nc.dram_tensor("gb", [128, NCH], mybir.dt.float32, kind="ExternalInput")
    outT = nc.dram_tensor("outT", [D, RC], mybir.dt.float32, kind="ExternalOutput")

    with tile.TileContext(nc) as tc:
        with (
            tc.tile_pool(name="consts", bufs=1) as consts,
            tc.tile_pool(name="xtp", bufs=1) as xtp,
            tc.tile_pool(name="cnn", bufs=1) as cnnp,
            tc.tile_pool(name="work", bufs=2) as work,
            tc.tile_pool(name="cpsum", bufs=2, space="PSUM") as cpsum,
            tc.tile_pool(name="gpsum", bufs=2, space="PSUM") as gpsum,
        ):
            cwr_sb = [None] * NCH
            xt_sb = [None] * NS

            def load_xt(g, j):
                t = xtp.tile([128, NCH * SCOL], mybir.dt.bfloat16, tag=f"xt{j}")
                nc.sync.dma_start(t[:], xt[g * NS + j])
                xt_sb[j] = t

            def load_cw_j(dck, j):
                # j-slice of cwr[dck] for fine-grained ramp
                if cwr_sb[dck] is None:
                    cwt = consts.tile([128, NS * NCH * 128],
                                      mybir.dt.bfloat16, tag=f"cw{dck}")
                    cwr_sb[dck] = cwt
                sl = slice(j * NCH * 128, (j + 1) * NCH * 128)
                nc.sync.dma_start(cwr_sb[dck][:, sl], cwr[dck][:, sl])

            # ramp: interleave (xt g0 j-chunk, cwr0 j-chunk) in first-use order
            for j in range(NS):
                load_xt(0, j)
                load_cw_j(0, j)
            cb_sb = consts.tile([128, NCH], mybir.dt.float32, tag="cb")
            nc.sync.dma_start(cb_sb[:], cb[:])
            gb_sb = consts.tile([128, NCH], mybir.dt.float32, tag="gb")
            nc.sync.dma_start(gb_sb[:], gb[:])
            for dck in range(1, NCH):
                cwt = consts.tile([128, NS * NCH * 128],
                                  mybir.dt.bfloat16, tag=f"cw{dck}")
                cwr_sb[dck] = cwt
                nc.sync.dma_start(cwt[:], cwr[dck])
            gwr_sb = []
            for eck in range(NCH):
                t = consts.tile([128, NCH * 128], mybir.dt.bfloat16, tag=f"gw{eck}")
                nc.sync.dma_start(t[:], gwr[eck])
                gwr_sb.append(t)

            # HAM warmup during the DMA ramp
            scr = consts.tile([128, 512], mybir.dt.bfloat16, tag="scr")
            nc.gpsimd.memset(scr[:], 0.0)
            for _ in range(16):
                wps = gpsum.tile([128, 480], mybir.dt.float32, tag="gps")
                nc.tensor.matmul(wps[:], scr[:, :128], scr[:, :480],
                                 start=True, stop=True)

            cnn_t = [[None] * NCH for _ in range(NG)]

            def conv_unit(g, dck):
                ps = cpsum.tile([128, NS * SCOL], mybir.dt.float32, tag="cps")
                for j in range(NS):
                    for ck in range(NCH):
                        nc.tensor.matmul(
                            ps[:, j * SCOL:(j + 1) * SCOL],
                            cwr_sb[dck][:, (j * NCH + ck) * 128:
                                        (j * NCH + ck + 1) * 128],
                            xt_sb[j][:, ck * SCOL:(ck + 1) * SCOL],
                            start=(ck == 0),
                            stop=(ck == NCH - 1),
                        )
                combine(g, dck, ps)

            def combine(g, dck, ps):
                AB = 2 * GWN                     # 192
                XL, XH = AB, AB + NT             # X block 192:224
                YH = XH + NT                     # Y block 224:256
                cbs = cb_sb[:, dck:dck + 1]
                bf = mybir.dt.bfloat16

                def m(j, lo=0, hi=SCOL):
                    return ps[:, j * SCOL + lo:j * SCOL + hi]

                def st(tag, n=SCOL):
                    return work.tile([128, n], bf, tag=tag, name=tag)

                cnn = cnnp.tile([128, GN], bf, tag=f"cnn{g}_{dck}")
                cnn_t[g][dck] = cnn
                v = cnn[:].rearrange("p (w t) -> p t w", t=W)

                def outab(t0):
                    # A-tile output t0 and B-tile output t0+4: [128, 2, 96]
                    return v[:, t0:t0 + 5:4]

                # ScalarE: scaled copies of m3/m4 (+conv bias), PSUM->SBUF
                u1 = st("u1"); nc.scalar.activation(u1[:], m(3), AF.Identity,
                                                    bias=cbs, scale=2.0)
                h1 = st("h1"); nc.scalar.activation(h1[:], m(4), AF.Identity,
                                                    scale=-0.5)
                u2 = st("u2", AB); nc.scalar.activation(u2[:], m(3, 0, AB),
                                                        AF.Identity, bias=cbs,
                                                        scale=4.0)
                h2 = st("h2", AB); nc.scalar.activation(h2[:], m(4, 0, AB),
                                                        AF.Identity, scale=0.25)
                u3 = st("u3", AB); nc.scalar.activation(u3[:], m(3, 0, AB),
                                                        AF.Identity, bias=cbs,
                                                        scale=8.0)
                h3 = st("h3", AB); nc.scalar.activation(h3[:], m(4, 0, AB),
                                                        AF.Identity, scale=-0.125)
                # evacuate m2/m4 so every tensor_tensor has <=1 PSUM operand
                c2s = st("c2s"); nc.scalar.activation(c2s[:], m(2), AF.Copy)
                c4s = st("c4s"); nc.scalar.activation(c4s[:], m(4), AF.Copy)
                # DVE: single-PSUM-operand combines
                s1 = st("s1"); nc.vector.tensor_add(s1[:], m(1), c2s[:])
                d1 = st("d1"); nc.vector.tensor_sub(d1[:], m(1), c2s[:])
                a0 = st("a0"); nc.vector.tensor_add(a0[:], m(3), c4s[:])
                P = st("P", XH)
                nc.vector.tensor_add(P[:], m(0, 0, XH), s1[:, :XH])
                t0 = st("t0")
                nc.scalar.activation(t0[:], a0[:], AF.Identity, bias=cbs)
                # GpSimd: contiguous SBUF-only combines
                w1 = st("w1"); nc.gpsimd.tensor_add(w1[:], d1[:], u1[:])
                w2 = st("w2", AB); nc.gpsimd.tensor_add(w2[:], s1[:, :AB], u2[:])
                w3 = st("w3", AB); nc.gpsimd.tensor_add(w3[:], d1[:, :AB], u3[:])
                x3 = st("x3", AB); nc.gpsimd.tensor_add(x3[:], w3[:], h3[:])
                v2 = st("v2", NT)
                nc.gpsimd.tensor_add(v2[:], u1[:, XL:XH], h1[:, XL:XH])
                # DVE: output writes (strided APs)
                nc.vector.tensor_add(outab(0), P[:, :AB], t0[:, :AB])     # y0
                nc.vector.tensor_add(outab(1), w1[:, :AB], h1[:, :AB])    # y1
                nc.vector.tensor_add(outab(2), w2[:], h2[:])              # y2
                nc.vector.tensor_add(outab(3), x3[:], m(5, 0, AB))        # y3
                # tails: T1 at w%3==0, T2 at w%3==1, T3 at w%3==2
                nc.scalar.activation(v[:, 8, 0::3], P[:, XL:XH], AF.Identity,
                                     bias=cbs)                            # y8 T1
                e1 = st("e1", NT)
                nc.vector.tensor_add(e1[:], d1[:, XL:XH], m(5, XL, XH))
                nc.scalar.activation(v[:, 9, 0::3], e1[:], AF.Identity,
                                     bias=cbs)                            # y9 T1
                nc.vector.tensor_add(v[:, 8, 1::3], m(0, XH, YH), t0[:, XL:XH])
                nc.vector.tensor_add(v[:, 9, 1::3], v2[:], m(5, XH, YH))  # y9 T2
                nc.vector.tensor_add(v[:, 8, 2::3], s1[:, XH:YH], t0[:, XH:YH])
                nc.vector.tensor_add(v[:, 9, 2::3], w1[:, XH:YH], h1[:, XH:YH])

            def gate_unit(g, eck, last=False):
                for c in range(2):
                    ps2 = gpsum.tile([128, 480], mybir.dt.float32, tag="gps")
                    for dck in range(NCH):
                        nc.tensor.matmul(
                            ps2[:],
                            gwr_sb[eck][:, dck * 128:(dck + 1) * 128],
                            cnn_t[g][dck][:, c * 480:(c + 1) * 480],
                            start=(dck == 0),
                            stop=(dck == NCH - 1),
                        )
                    gt = work.tile([128, 480], mybir.dt.bfloat16, tag="gate")
                    ot = work.tile([128, 480], mybir.dt.float32, tag="out")
                    chunks = ((0, 240), (240, 480)) if (last and c == 1) \
                        else ((0, 480),)
                    for lo, hi in chunks:
                        nc.scalar.activation(gt[:, lo:hi], ps2[:, lo:hi],
                                             AF.Sigmoid,
                                             bias=gb_sb[:, eck:eck + 1])
                        nc.vector.tensor_mul(ot[:, lo:hi],
                                             cnn_t[g][eck][:, c * 480 + lo:
                                                           c * 480 + hi],
                                             gt[:, lo:hi])
                        nc.sync.dma_start(
                            outT[eck * 128:(eck + 1) * 128,
                                 g * GN + c * 480 + lo:g * GN + c * 480 + hi],
                            ot[:, lo:hi])

            # phase g0 convs (cwr streams behind; xt g0 in j-chunks)
            for dck in range(NCH):
                conv_unit(0, dck)
            # g0 gates; xt g1 load issued after the first gate's outputs
            gate_unit(0, 0)
            for j in range(NS):
                load_xt(1, j)
            for eck in range(1, NCH):
                gate_unit(0, eck)
            # phase g1 convs + gates
            for dck in range(NCH):
                conv_unit(1, dck)
            for eck in range(NCH):
                gate_unit(1, eck, last=(eck == NCH - 1))
    nc.compile()
    return nc


def _stream_weights(conv_w):
    W0, W1, W2 = [conv_w[:, :, k].astype(np.float64) for k in range(3)]
    g = []
    for b in PTS:
        g.append(W2 if b is INF else W0 + b * W1 + b * b * W2)
    return np.stack(g)  # [6, Dout, Din]


def _prep_weights(conv_w, conv_b, gate_w, gate_b):
    garr = _stream_weights(conv_w)                       # [6, Do, Di]
    # cwr[dck][cc, (j*8+ck)*128+dd] = g_j[dck*128+dd, ck*128+cc]
    gv = garr.reshape(NS, NCH, 128, NCH, 128)            # [j, dck, dd, ck, cc]
    cw_host = np.ascontiguousarray(gv.transpose(1, 4, 0, 3, 2)).reshape(
        NCH, 128, NS * NCH * 128).astype(BF16)
    gwt = gate_w.T.reshape(NCH, 128, NCH, 128)           # [dck, dd, eck, ee]
    gw_host = np.ascontiguousarray(gwt.transpose(2, 1, 0, 3)).reshape(
        NCH, 128, NCH * 128).astype(BF16)
    cb_host = np.ascontiguousarray(conv_b.reshape(NCH, 128).T).astype(np.float32)
    gb_host = np.ascontiguousarray(gate_b.reshape(NCH, 128).T).astype(np.float32)
    return cw_host, gw_host, cb_host, gb_host


def _prep_core_x(x_shard):
    # x_shard [BC, T, D] -> xt [NG*6, 128, NCH*SCOL]
    xw = x_shard.reshape(NWIN, W, D).astype(np.float64)
    xp = np.pad(xw, ((0, 0), (1, 1), (0, 0)))            # [192, 12, D]
    xt_host = np.empty((NG * NS, 128, NCH * SCOL), BF16)
    for g in range(NG):
        ws = xp[g * GWN:(g + 1) * GWN]
        xA = np.einsum('ji,wic->jwc', BA, ws[:, 0:6])    # [6, 96, D]
        xB = np.einsum('ji,wic->jwc', BA, ws[:, 4:10])
        xTl = []
        for ci, (pts, mp) in enumerate(TAIL):
            wc = ws[ci::3][:, 8:12]                      # [32, 4, D]
            xTl.append(np.einsum('ji,wic->jwc', BT[ci], wc))
        for j in range(NS):
            cX, cY = XY[j]
            jX = TAIL[cX][1].index(j)
            jY = TAIL[cY][1].index(j)
            S = np.concatenate([xA[j], xB[j], xTl[cX][jX], xTl[cY][jY]],
                               axis=0)                   # [256, D]
            blk = S.T.reshape(NCH, 128, SCOL).transpose(1, 0, 2)
            xt_host[g * NS + j] = blk.reshape(128, NCH * SCOL).astype(BF16)
    return xt_host


def _unshard_core(o):
    # o: [D, RC] cols ordered (g, w, t) -> [BC, T, D]
    return np.ascontiguousarray(
        o.reshape(D, NG, GWN, W).transpose(1, 2, 3, 0).reshape(BC, T, D))


_NC_CACHE = None


def _prep_in_maps(x, conv_w, conv_b, gate_w, gate_b):
    cw_host, gw_host, cb_host, gb_host = _prep_weights(
        conv_w, conv_b, gate_w, gate_b)
    return [
        {"xt": _prep_core_x(x[BC * i:BC * (i + 1)]), "cwr": cw_host,
         "gwr": gw_host, "cb": cb_host, "gb": gb_host}
        for i in range(NCORES)
    ]


def kernel(x, conv_w, conv_b, gate_w, gate_b):
    global _NC_CACHE
    x = np.asarray(x, np.float32)
    conv_w = np.asarray(conv_w, np.float32)
    conv_b = np.asarray(conv_b, np.float32)
    gate_w = np.asarray(gate_w, np.float32)
    gate_b = np.asarray(gate_b, np.float32)

    in_maps = _prep_in_maps(x, conv_w, conv_b, gate_w, gate_b)
    if _NC_CACHE is None:
        _NC_CACHE = _build()
    res = run_bass_kernel_spmd(_NC_CACHE, in_maps,
                               core_ids=list(range(NCORES))).results

    out = np.empty((B, T, D), np.float32)
    for i in range(NCORES):
        out[BC * i:BC * (i + 1)] = _unshard_core(np.asarray(res[i]["outT"]))
    return out


# revision 10
# speedup vs baseline: 1.1296x; 1.1296x over previous
"""Trainium2 Bass kernel for windowed Conv1d(k=3) + sigmoid gating.

Reference (B=16, T=960, D=1024, W=10): windows of 10 conv'd independently
with per-window zero pad 1:
    cnn[t, d] = sum_{k,c} conv_w[d, c, k] * xpad[t + k, c]
    out = cnn * sigmoid(cnn @ gate_w.T + gate_b)

Strategy: data parallel over 8 cores (2 batches / 192 windows / core).
The conv uses hybrid Winograd: two F(4,3) tiles (outputs 0-3 from
xp[0:6], outputs 4-7 from xp[4:10]) at points {0,1,-1,2,-1/2,inf} plus an
F(2,3) tail (outputs 8,9 from xp[8:12]) whose 4 products FOLD into the
same 6 weight matrices by choosing the tail's interpolation points as a
subset of the F43 points.  Rotating the tail point-set over 3 window
classes (w%3) equalizes the streams:
    T1=(0,1,-1,inf)->streams(0,1,2,5)  T2=(0,2,-1/2,inf)->(0,3,4,5)
    T3=(1,-1,2,-1/2)->(1,2,3,4)
so every stream carries exactly 8 columns per 3 windows: 16 muls/window
(vs 30 direct, 20 for F(2,3)) => conv PE time ~82us + gate ~52us.

Streams use canonical Vandermonde weights g_b = [1,b,b^2] . W (the
normalization lives in the host-side input transforms V^{-T}).  Per core:
2 groups of 96 windows; per (group, dck): 6 accumulation chains of N=256
(PSUM: 3 banks/slot, 2 slots + 2 gate banks = 8).  The A^T combine runs
on ScalarE/VectorE/GpSimd under the matmul stream with bf16 intermediates.
Host does padding, transposition to channel-major, input transforms, and
weight transforms (f64 -> bf16).
"""

import numpy as np
import ml_dtypes

import concourse.bacc as bacc
import concourse.bass as bass
import concourse.tile as tile
from concourse import mybir
from concourse.bass_utils import run_bass_kernel_spmd

BF16 = ml_dtypes.bfloat16

B, T, D, W = 16, 960, 1024, 10
NCORES = 8
BC = B // NCORES             # batches per core (2)
NWIN = BC * T // W           # windows per core (192)
RC = NWIN * W                # output rows per core (1920)
NG = 2                       # groups per core
GWN = NWIN // NG             # windows per group (96)
GN = GWN * W                 # output cols per group (960)
NS = 6                       # winograd streams
SCOL = GWN * 16 // 6         # stream cols per group (256)
NCH = D // 128               # d chunks (8)
NT = GWN // 3                # tail windows per class per group (32)
AF = mybir.ActivationFunctionType

INF = "inf"
PTS = [0.0, 1.0, -1.0, 2.0, -0.5, INF]
T1_PTS = [0.0, 1.0, -1.0, INF]; T1_MAP = [0, 1, 2, 5]
T2_PTS = [0.0, 2.0, -0.5, INF]; T2_MAP = [0, 3, 4, 5]
T3_PTS = [1.0, -1.0, 2.0, -0.5]; T3_MAP = [1, 2, 3, 4]
# class feeding stream j at slot X / Y (class index 0/1/2 = T1/T2/T3)
XY = {0: (0, 1), 1: (0, 2), 2: (0, 2), 3: (1, 2), 4: (1, 2), 5: (0, 1)}
TAIL = [(T1_PTS, T1_MAP), (T2_PTS, T2_MAP), (T3_PTS, T3_MAP)]


def _vinv_T(points):
    n = len(points)
    V = np.zeros((n, n))
    for j, b in enumerate(points):
        if b is INF:
            V[j, n - 1] = 1.0
        else:
            V[j] = [b ** i for i in range(n)]
    return np.linalg.inv(V).T


BA = _vinv_T(PTS)
BT = [_vinv_T(p) for p, _ in TAIL]


def _build():
    nc = bacc.Bacc("TRN2", target_bir_lowering=False, debug=False)

    # xt[g*6+j]: [cc, (ck, col)] transformed input, one DMA per (g, j)
    xt = nc.dram_tensor("xt", [NG * NS, 128, NCH * SCOL], mybir.dt.bfloat16,
                        kind="ExternalInput")
    # cwr[dck]: [cc, ((j*NCH+ck)*128 + dd)] conv lhsT blocks
    cwr = nc.dram_tensor("cwr", [NCH, 128, NS * NCH * 128], mybir.dt.bfloat16,
                         kind="ExternalInput")
    # gwr[eck]: [dd, (dck*128 + ee)] gate lhsT blocks
    gwr = nc.dram_tensor("gwr", [NCH, 128, NCH * 128], mybir.dt.bfloat16,
                         kind="ExternalInput")
    cb = nc.dram_tensor("cb", [128, NCH], mybir.dt.float32, kind="ExternalInput")
    gb = nc.dram_tensor("gb", [128, NCH], mybir.dt.float32, kind="ExternalInput")
    outT = nc.dram_tensor("outT", [D, RC], mybir.dt.float32, kind="ExternalOutput")

    with tile.TileContext(nc) as tc:
        with (
            tc.tile_pool(name="consts", bufs=1) as consts,
            tc.tile_pool(name="xtp", bufs=1) as xtp,
            tc.tile_pool(name="cnn", bufs=1) as cnnp,
            tc.tile_pool(name="work", bufs=2) as work,
            tc.tile_pool(name="cpsum", bufs=2, space="PSUM") as cpsum,
            tc.tile_pool(name="gpsum", bufs=2, space="PSUM") as gpsum,
        ):
            cwr_sb = [None] * NCH
            xt_sb = [None] * NS

            def load_xt(g, j):
                t = xtp.tile([128, NCH * SCOL], mybir.dt.bfloat16, tag=f"xt{j}")
                nc.sync.dma_start(t[:], xt[g * NS + j])
                xt_sb[j] = t

            def load_cw_j(dck, j):
                # j-slice of cwr[dck] for fine-grained ramp
                if cwr_sb[dck] is None:
                    cwt = consts.tile([128, NS * NCH * 128],
                                      mybir.dt.bfloat16, tag=f"cw{dck}")
                    cwr_sb[dck] = cwt
                sl = slice(j * NCH * 128, (j + 1) * NCH * 128)
                nc.sync.dma_start(cwr_sb[dck][:, sl], cwr[dck][:, sl])

            # ramp: interleave (xt g0 j-chunk, cwr0 j-chunk) in first-use order
            for j in range(NS):
                load_xt(0, j)
                load_cw_j(0, j)
            cb_sb = consts.tile([128, NCH], mybir.dt.float32, tag="cb")
            nc.sync.dma_start(cb_sb[:], cb[:])
            gb_sb = consts.tile([128, NCH], mybir.dt.float32, tag="gb")
            nc.sync.dma_start(gb_sb[:], gb[:])
            for dck in range(1, NCH):
                cwt = consts.tile([128, NS * NCH * 128],
                                  mybir.dt.bfloat16, tag=f"cw{dck}")
                cwr_sb[dck] = cwt
                nc.sync.dma_start(cwt[:], cwr[dck])
            gwr_sb = []
            for eck in range(NCH):
                t = consts.tile([128, NCH * 128], mybir.dt.bfloat16, tag=f"gw{eck}")
                nc.sync.dma_start(t[:], gwr[eck])
                gwr_sb.append(t)

            # HAM warmup during the DMA ramp
            scr = consts.tile([128, 512], mybir.dt.bfloat16, tag="scr")
            nc.gpsimd.memset(scr[:], 0.0)
            for _ in range(16):
                wps = gpsum.tile([128, 480], mybir.dt.float32, tag="gps")
                nc.tensor.matmul(wps[:], scr[:, :128], scr[:, :480],
                                 start=True, stop=True)

            cnn_t = [[None] * NCH for _ in range(NG)]

            def conv_unit(g, dck):
                ps = cpsum.tile([128, NS * SCOL], mybir.dt.float32, tag="cps")
                for j in range(NS):
                    for ck in range(NCH):
                        nc.tensor.matmul(
                            ps[:, j * SCOL:(j + 1) * SCOL],
                            cwr_sb[dck][:, (j * NCH + ck) * 128:
                                        (j * NCH + ck + 1) * 128],
                            xt_sb[j][:, ck * SCOL:(ck + 1) * SCOL],
                            start=(ck == 0),
                            stop=(ck == NCH - 1),
                        )
                combine(g, dck, ps)

            def combine(g, dck, ps):
                AB = 2 * GWN                     # 192
                XL, XH = AB, AB + NT             # X block 192:224
                YH = XH + NT                     # Y block 224:256
                cbs = cb_sb[:, dck:dck + 1]
                bf = mybir.dt.bfloat16

                def m(j, lo=0, hi=SCOL):
                    return ps[:, j * SCOL + lo:j * SCOL + hi]

                def st(tag, n=SCOL):
                    return work.tile([128, n], bf, tag=tag, name=tag)

                # cnn cols = t*96 + w' (t-major; w' = cls*32 + w//3) so every
                # combine write below is contiguous in the free dim
                cnn = cnnp.tile([128, GN], bf, tag=f"cnn{g}_{dck}")
                cnn_t[g][dck] = cnn
                v = cnn[:].rearrange("p (t w) -> p t w", w=GWN)

                def outab(t0):
                    # A-tile output t0 and B-tile output t0+4: [128, 2, 96]
                    return v[:, t0:t0 + 5:4]

                # ScalarE: scaled copies of m3/m4 (+conv bias), PSUM->SBUF
                u1 = st("u1"); nc.scalar.activation(u1[:], m(3), AF.Identity,
                                                    bias=cbs, scale=2.0)
                h1 = st("h1"); nc.scalar.activation(h1[:], m(4), AF.Identity,
                                                    scale=-0.5)
                u2 = st("u2", AB); nc.scalar.activation(u2[:], m(3, 0, AB),
                                                        AF.Identity, bias=cbs,
                                                        scale=4.0)
                h2 = st("h2", AB); nc.scalar.activation(h2[:], m(4, 0, AB),
                                                        AF.Identity, scale=0.25)
                u3 = st("u3", AB); nc.scalar.activation(u3[:], m(3, 0, AB),
                                                        AF.Identity, bias=cbs,
                                                        scale=8.0)
                h3 = st("h3", AB); nc.scalar.activation(h3[:], m(4, 0, AB),
                                                        AF.Identity, scale=-0.125)
                # evacuate m2/m4 so every tensor_tensor has <=1 PSUM operand
                c2s = st("c2s"); nc.scalar.activation(c2s[:], m(2), AF.Copy)
                c4s = st("c4s"); nc.scalar.activation(c4s[:], m(4), AF.Copy)
                # DVE: single-PSUM-operand combines
                s1 = st("s1"); nc.vector.tensor_add(s1[:], m(1), c2s[:])
                d1 = st("d1"); nc.vector.tensor_sub(d1[:], m(1), c2s[:])
                a0 = st("a0"); nc.vector.tensor_add(a0[:], m(3), c4s[:])
                P = st("P", XH)
                nc.vector.tensor_add(P[:], m(0, 0, XH), s1[:, :XH])
                t0 = st("t0")
                nc.scalar.activation(t0[:], a0[:], AF.Identity, bias=cbs)
                # GpSimd: contiguous SBUF-only combines + output writes
                w1 = st("w1"); nc.gpsimd.tensor_add(w1[:], d1[:], u1[:])
                w2 = st("w2", AB); nc.gpsimd.tensor_add(w2[:], s1[:, :AB], u2[:])
                w3 = st("w3", AB); nc.gpsimd.tensor_add(w3[:], d1[:, :AB], u3[:])
                x3 = st("x3", AB); nc.gpsimd.tensor_add(x3[:], w3[:], h3[:])
                v2 = st("v2", NT)
                nc.gpsimd.tensor_add(v2[:], u1[:, XL:XH], h1[:, XL:XH])
                nc.gpsimd.tensor_add(outab(0), P[:, :AB], t0[:, :AB])     # y0
                nc.gpsimd.tensor_add(outab(1), w1[:, :AB], h1[:, :AB])    # y1
                nc.gpsimd.tensor_add(outab(2), w2[:], h2[:])              # y2
                nc.vector.tensor_add(outab(3), x3[:], m(5, 0, AB))        # y3
                # tails: T1 -> w' 0:32, T2 -> 32:64, T3 -> 64:96
                nc.vector.tensor_scalar_add(v[:, 8, 0:NT], P[:, XL:XH], cbs)
                e1 = st("e1", NT)
                nc.vector.tensor_add(e1[:], d1[:, XL:XH], m(5, XL, XH))
                nc.vector.tensor_scalar_add(v[:, 9, 0:NT], e1[:], cbs)    # y9 T1
                nc.vector.tensor_add(v[:, 8, NT:2 * NT], m(0, XH, YH),
                                     t0[:, XL:XH])                        # y8 T2
                nc.vector.tensor_add(v[:, 9, NT:2 * NT], v2[:], m(5, XH, YH))
                nc.gpsimd.tensor_add(v[:, 8, 2 * NT:], s1[:, XH:YH],
                                     t0[:, XH:YH])                        # y8 T3
                nc.gpsimd.tensor_add(v[:, 9, 2 * NT:], w1[:, XH:YH],
                                     h1[:, XH:YH])                        # y9 T3

            def gate_unit(g, eck, last=False):
                for c in range(2):
                    ps2 = gpsum.tile([128, 480], mybir.dt.float32, tag="gps")
                    for dck in range(NCH):
                        nc.tensor.matmul(
                            ps2[:],
                            gwr_sb[eck][:, dck * 128:(dck + 1) * 128],
                            cnn_t[g][dck][:, c * 480:(c + 1) * 480],
                            start=(dck == 0),
                            stop=(dck == NCH - 1),
                        )
                    gt = work.tile([128, 480], mybir.dt.bfloat16, tag="gate")
                    ot = work.tile([128, 480], mybir.dt.float32, tag="out")
                    chunks = ((0, 240), (240, 480)) if (last and c == 1) \
                        else ((0, 480),)
                    for lo, hi in chunks:
                        nc.scalar.activation(gt[:, lo:hi], ps2[:, lo:hi],
                                             AF.Sigmoid,
                                             bias=gb_sb[:, eck:eck + 1])
                        nc.vector.tensor_mul(ot[:, lo:hi],
                                             cnn_t[g][eck][:, c * 480 + lo:
                                                           c * 480 + hi],
                                             gt[:, lo:hi])
                        nc.sync.dma_start(
                            outT[eck * 128:(eck + 1) * 128,
                                 g * GN + c * 480 + lo:g * GN + c * 480 + hi],
                            ot[:, lo:hi])

            # phase g0 convs (cwr streams behind; xt g0 in j-chunks)
            for dck in range(NCH):
                conv_unit(0, dck)
            # g0 gates; xt g1 load issued after the first gate's outputs
            gate_unit(0, 0)
            for j in range(NS):
                load_xt(1, j)
            for eck in range(1, NCH):
                gate_unit(0, eck)
            # phase g1 convs + gates
            for dck in range(NCH):
                conv_unit(1, dck)
            for eck in range(NCH):
                gate_unit(1, eck, last=(eck == NCH - 1))
    nc.compile()
    return nc


def _stream_weights(conv_w):
    W0, W1, W2 = [conv_w[:, :, k].astype(np.float64) for k in range(3)]
    g = []
    for b in PTS:
        g.append(W2 if b is INF else W0 + b * W1 + b * b * W2)
    return np.stack(g)  # [6, Dout, Din]


def _prep_weights(conv_w, conv_b, gate_w, gate_b):
    garr = _stream_weights(conv_w)                       # [6, Do, Di]
    # cwr[dck][cc, (j*8+ck)*128+dd] = g_j[dck*128+dd, ck*128+cc]
    gv = garr.reshape(NS, NCH, 128, NCH, 128)            # [j, dck, dd, ck, cc]
    cw_host = np.ascontiguousarray(gv.transpose(1, 4, 0, 3, 2)).reshape(
        NCH, 128, NS * NCH * 128).astype(BF16)
    gwt = gate_w.T.reshape(NCH, 128, NCH, 128)           # [dck, dd, eck, ee]
    gw_host = np.ascontiguousarray(gwt.transpose(2, 1, 0, 3)).reshape(
        NCH, 128, NCH * 128).astype(BF16)
    cb_host = np.ascontiguousarray(conv_b.reshape(NCH, 128).T).astype(np.float32)
    gb_host = np.ascontiguousarray(gate_b.reshape(NCH, 128).T).astype(np.float32)
    return cw_host, gw_host, cb_host, gb_host


def _prep_core_x(x_shard):
    # x_shard [BC, T, D] -> xt [NG*6, 128, NCH*SCOL]
    xw = x_shard.reshape(NWIN, W, D).astype(np.float64)
    xp = np.pad(xw, ((0, 0), (1, 1), (0, 0)))            # [192, 12, D]
    xt_host = np.empty((NG * NS, 128, NCH * SCOL), BF16)
    # class-major window order within a group: w' = cls*32 + w//3
    perm = np.array([3 * wi + cls for cls in range(3) for wi in range(NT)])
    for g in range(NG):
        ws = xp[g * GWN:(g + 1) * GWN]
        xA = np.einsum('ji,wic->jwc', BA, ws[perm][:, 0:6])  # [6, 96, D]
        xB = np.einsum('ji,wic->jwc', BA, ws[perm][:, 4:10])
        xTl = []
        for ci, (pts, mp) in enumerate(TAIL):
            wc = ws[ci::3][:, 8:12]                      # [32, 4, D]
            xTl.append(np.einsum('ji,wic->jwc', BT[ci], wc))
        for j in range(NS):
            cX, cY = XY[j]
            jX = TAIL[cX][1].index(j)
            jY = TAIL[cY][1].index(j)
            S = np.concatenate([xA[j], xB[j], xTl[cX][jX], xTl[cY][jY]],
                               axis=0)                   # [256, D]
            blk = S.T.reshape(NCH, 128, SCOL).transpose(1, 0, 2)
            xt_host[g * NS + j] = blk.reshape(128, NCH * SCOL).astype(BF16)
    return xt_host


def _unshard_core(o):
    # o: [D, RC] cols ordered (g, t, cls, widx); window w = 3*widx + cls
    return np.ascontiguousarray(
        o.reshape(D, NG, W, 3, NT).transpose(1, 4, 3, 2, 0).reshape(BC, T, D))


_NC_CACHE = None


def _prep_in_maps(x, conv_w, conv_b, gate_w, gate_b):
    cw_host, gw_host, cb_host, gb_host = _prep_weights(
        conv_w, conv_b, gate_w, gate_b)
    return [
        {"xt": _prep_core_x(x[BC * i:BC * (i + 1)]), "cwr": cw_host,
         "gwr": gw_host, "cb": cb_host, "gb": gb_host}
        for i in range(NCORES)
    ]


def kernel(x, conv_w, conv_b, gate_w, gate_b):
    global _NC_CACHE
    x = np.asarray(x, np.float32)
    conv_w = np.asarray(conv_w, np.float32)
    conv_b = np.asarray(conv_b, np.float32)
    gate_w = np.asarray(gate_w, np.float32)
    gate_b = np.asarray(gate_b, np.float32)

    in_maps = _prep_in_maps(x, conv_w, conv_b, gate_w, gate_b)
    if _NC_CACHE is None:
        _NC_CACHE = _build()
    res = run_bass_kernel_spmd(_NC_CACHE, in_maps,
                               core_ids=list(range(NCORES))).results

    out = np.empty((B, T, D), np.float32)
    for i in range(NCORES):
        out[BC * i:BC * (i + 1)] = _unshard_core(np.asarray(res[i]["outT"]))
    return out


# revision 18
# speedup vs baseline: 1.2089x; 1.0702x over previous
"""Trainium2 Bass kernel for windowed Conv1d(k=3) + sigmoid gating.

Reference (B=16, T=960, D=1024, W=10): windows of 10 conv'd independently
with per-window zero pad 1:
    cnn[t, d] = sum_{k,c} conv_w[d, c, k] * xpad[t + k, c]
    out = cnn * sigmoid(cnn @ gate_w.T + gate_b)

Strategy: data parallel over 8 cores (2 batches / 192 windows / core).
The conv uses hybrid Winograd: two F(4,3) tiles (outputs 0-3 from
xp[0:6], outputs 4-7 from xp[4:10]) at points {0,1,-1,2,-1/2,inf} plus an
F(2,3) tail (outputs 8,9 from xp[8:12]) whose 4 products FOLD into the
same 6 weight matrices by choosing the tail's interpolation points as a
subset of the F43 points.  Rotating the tail point-set over 3 window
classes (w%3) equalizes the streams:
    T1=(0,1,-1,inf)->streams(0,1,2,5)  T2=(0,2,-1/2,inf)->(0,3,4,5)
    T3=(1,-1,2,-1/2)->(1,2,3,4)
so every stream carries exactly 8 columns per 3 windows: 16 muls/window
(vs 30 direct, 20 for F(2,3)) => conv PE time ~82us + gate ~52us.

Streams use canonical Vandermonde weights g_b = [1,b,b^2] . W (the
normalization lives in the host-side input transforms V^{-T}).  Per core:
2 groups of 96 windows; per (group, dck): 6 accumulation chains of N=256
(PSUM: 3 banks/slot, 2 slots + 2 gate banks = 8).  The A^T combine runs
on ScalarE/VectorE/GpSimd under the matmul stream with bf16 intermediates.
Host does padding, transposition to channel-major, input transforms, and
weight transforms (f64 -> bf16).
"""

import numpy as np
import ml_dtypes

import concourse.bacc as bacc
import concourse.bass as bass
import concourse.tile as tile
from concourse import mybir
from concourse.bass_utils import run_bass_kernel_spmd

BF16 = ml_dtypes.bfloat16

B, T, D, W = 16, 960, 1024, 10
NCORES = 8
BC = B // NCORES             # batches per core (2)
NWIN = BC * T // W           # windows per core (192)
RC = NWIN * W                # output rows per core (1920)
NG = 2                       # groups per core
GWN = NWIN // NG             # windows per group (96)
GN = GWN * W                 # output cols per group (960)
NS = 6                       # winograd streams
SCOL = GWN * 16 // 6         # stream cols per group (256)
NCH = D // 128               # d chunks (8)
NT = GWN // 3                # tail windows per class per group (32)
AF = mybir.ActivationFunctionType

INF = "inf"
PTS = [0.0, 1.0, -1.0, 2.0, -0.5, INF]
T1_PTS = [0.0, 1.0, -1.0, INF]; T1_MAP = [0, 1, 2, 5]
T2_PTS = [0.0, 2.0, -0.5, INF]; T2_MAP = [0, 3, 4, 5]
T3_PTS = [1.0, -1.0, 2.0, -0.5]; T3_MAP = [1, 2, 3, 4]
# class feeding stream j at slot X / Y (class index 0/1/2 = T1/T2/T3)
XY = {0: (0, 1), 1: (0, 2), 2: (0, 2), 3: (1, 2), 4: (1, 2), 5: (0, 1)}
TAIL = [(T1_PTS, T1_MAP), (T2_PTS, T2_MAP), (T3_PTS, T3_MAP)]


def _vinv_T(points):
    n = len(points)
    V = np.zeros((n, n))
    for j, b in enumerate(points):
        if b is INF:
            V[j, n - 1] = 1.0
        else:
            V[j] = [b ** i for i in range(n)]
    return np.linalg.inv(V).T


BA = _vinv_T(PTS)
BT = [_vinv_T(p) for p, _ in TAIL]


# storage order of streams (chains emitted 0..5 in storage order): logical
# stream L[s]; m1/m2 first so the combine can start mid-unit
SJ = [1, 2, 3, 4, 0, 5]
SP = {j: s for s, j in enumerate(SJ)}   # logical -> storage


def _build():
    nc = bacc.Bacc("TRN2", target_bir_lowering=False, debug=False)

    # xt[g*2+h]: [cc, (s_local, ck, col)] transformed input, storage-stream
    # triples per half -> 12KB rows per DMA
    xt = nc.dram_tensor("xt", [NG * 2, 128, 3 * NCH * SCOL], mybir.dt.bfloat16,
                        kind="ExternalInput")
    # cwr[dck]: [cc, ((j*NCH+ck)*128 + dd)] conv lhsT blocks
    cwr = nc.dram_tensor("cwr", [NCH, 128, NS * NCH * 128], mybir.dt.bfloat16,
                         kind="ExternalInput")
    # gwr[eck]: [dd, (dck*128 + ee)] gate lhsT blocks
    gwr = nc.dram_tensor("gwr", [NCH, 128, NCH * 128], mybir.dt.bfloat16,
                         kind="ExternalInput")
    cb = nc.dram_tensor("cb", [128, NCH], mybir.dt.float32, kind="ExternalInput")
    gb = nc.dram_tensor("gb", [128, NCH], mybir.dt.float32, kind="ExternalInput")
    outT = nc.dram_tensor("outT", [D, RC], mybir.dt.float32, kind="ExternalOutput")

    with tile.TileContext(nc) as tc:
        with (
            tc.tile_pool(name="consts", bufs=1) as consts,
            tc.tile_pool(name="xtp", bufs=1) as xtp,
            tc.tile_pool(name="cnn", bufs=1) as cnnp,
            tc.tile_pool(name="work", bufs=2) as work,
            tc.tile_pool(name="cpsum", bufs=2, space="PSUM") as cpsum,
            tc.tile_pool(name="gpsum", bufs=2, space="PSUM") as gpsum,
        ):
            cwr_sb = [None] * NCH
            xt_sb = [None, None]        # per half

            def load_xt_half(g, h):
                t = xtp.tile([128, 3 * NCH * SCOL], mybir.dt.bfloat16,
                             tag=f"xt{h}")
                nc.sync.dma_start(t[:], xt[g * 2 + h])
                xt_sb[h] = t

            def cw_tile(dck):
                cwt = consts.tile([128, NS * NCH * 128],
                                  mybir.dt.bfloat16, tag=f"cw{dck}")
                cwr_sb[dck] = cwt
                return cwt

            # ramp: half-group interleave in first-use order
            cw0 = cw_tile(0)
            HB = 3 * NCH * 128
            load_xt_half(0, 0)
            nc.sync.dma_start(cw0[:, :HB], cwr[0][:, :HB])
            load_xt_half(0, 1)
            nc.sync.dma_start(cw0[:, HB:], cwr[0][:, HB:])
            cb_sb = consts.tile([128, NCH], mybir.dt.float32, tag="cb")
            nc.sync.dma_start(cb_sb[:], cb[:])
            gb_sb = consts.tile([128, NCH], mybir.dt.float32, tag="gb")
            nc.sync.dma_start(gb_sb[:], gb[:])
            for dck in range(1, NCH):
                nc.sync.dma_start(cw_tile(dck)[:], cwr[dck])
            gwr_sb = []
            for eck in range(NCH):
                t = consts.tile([128, NCH * 128], mybir.dt.bfloat16, tag=f"gw{eck}")
                nc.sync.dma_start(t[:], gwr[eck])
                gwr_sb.append(t)

            # HAM warmup during the DMA ramp
            scr = consts.tile([128, 512], mybir.dt.bfloat16, tag="scr")
            nc.gpsimd.memset(scr[:], 0.0)
            for _ in range(16):
                wps = gpsum.tile([128, 480], mybir.dt.float32, tag="gps")
                nc.tensor.matmul(wps[:], scr[:, :128], scr[:, :480],
                                 start=True, stop=True)

            cnn_t = [[None] * NCH for _ in range(NG)]

            def conv_unit(g, dck):
                ps = cpsum.tile([128, NS * SCOL], mybir.dt.float32, tag="cps")
                for s in range(NS):
                    h, sl = s // 3, s % 3
                    for ck in range(NCH):
                        nc.tensor.matmul(
                            ps[:, s * SCOL:(s + 1) * SCOL],
                            cwr_sb[dck][:, (s * NCH + ck) * 128:
                                        (s * NCH + ck + 1) * 128],
                            xt_sb[h][:, (sl * NCH + ck) * SCOL:
                                     (sl * NCH + ck + 1) * SCOL],
                            start=(ck == 0),
                            stop=(ck == NCH - 1),
                        )
                combine(g, dck, ps)

            def combine(g, dck, ps):
                AB = 2 * GWN                     # 192
                XL, XH = AB, AB + NT             # X block 192:224
                YH = XH + NT                     # Y block 224:256
                cbs = cb_sb[:, dck:dck + 1]
                bf = mybir.dt.bfloat16

                def m(j, lo=0, hi=SCOL):
                    s = SP[j]
                    return ps[:, s * SCOL + lo:s * SCOL + hi]

                def st(tag, n=SCOL):
                    return work.tile([128, n], bf, tag=tag, name=tag)

                # cnn cols = t*96 + w' (t-major; w' = cls*32 + w//3) so every
                # combine write below is contiguous in the free dim
                cnn = cnnp.tile([128, GN], bf, tag=f"cnn{g}_{dck}")
                cnn_t[g][dck] = cnn
                v = cnn[:].rearrange("p (t w) -> p t w", w=GWN)

                def outab(t0):
                    # A-tile output t0 and B-tile output t0+4: [128, 2, 96]
                    return v[:, t0:t0 + 5:4]

                # in chain availability order: m1, m2 first, then m3, m4,
                # m0, m5 (storage order SJ) so the combine overlaps the unit
                c2s = st("c2s"); nc.scalar.activation(c2s[:], m(2), AF.Copy)
                s1 = st("s1"); nc.vector.tensor_add(s1[:], m(1), c2s[:])
                d1 = st("d1"); nc.vector.tensor_sub(d1[:], m(1), c2s[:])
                c4s = st("c4s"); nc.scalar.activation(c4s[:], m(4), AF.Copy)
                u1 = st("u1"); nc.scalar.activation(u1[:], m(3), AF.Identity,
                                                    bias=cbs, scale=2.0)
                h1 = st("h1"); nc.scalar.activation(h1[:], m(4), AF.Identity,
                                                    scale=-0.5)
                u2 = st("u2", AB); nc.scalar.activation(u2[:], m(3, 0, AB),
                                                        AF.Identity, bias=cbs,
                                                        scale=4.0)
                h2 = st("h2", AB); nc.scalar.activation(h2[:], m(4, 0, AB),
                                                        AF.Identity, scale=0.25)
                u3 = st("u3", AB); nc.scalar.activation(u3[:], m(3, 0, AB),
                                                        AF.Identity, bias=cbs,
                                                        scale=8.0)
                h3 = st("h3", AB); nc.scalar.activation(h3[:], m(4, 0, AB),
                                                        AF.Identity, scale=-0.125)
                a0 = st("a0"); nc.vector.tensor_add(a0[:], m(3), c4s[:])
                t0 = st("t0"); nc.vector.tensor_scalar_add(t0[:], a0[:], cbs)
                w1 = st("w1"); nc.gpsimd.tensor_add(w1[:], d1[:], u1[:])
                w2 = st("w2", AB); nc.gpsimd.tensor_add(w2[:], s1[:, :AB], u2[:])
                w3 = st("w3", AB); nc.gpsimd.tensor_add(w3[:], d1[:, :AB], u3[:])
                x3 = st("x3", AB); nc.gpsimd.tensor_add(x3[:], w3[:], h3[:])
                v2 = st("v2", NT)
                nc.vector.tensor_add(v2[:], u1[:, XL:XH], h1[:, XL:XH])
                P = st("P", XH)
                nc.vector.tensor_add(P[:], m(0, 0, XH), s1[:, :XH])
                nc.gpsimd.tensor_add(outab(0), P[:, :AB], t0[:, :AB])     # y0
                nc.gpsimd.tensor_add(outab(1), w1[:, :AB], h1[:, :AB])    # y1
                nc.gpsimd.tensor_add(outab(2), w2[:], h2[:])              # y2
                nc.vector.tensor_add(outab(3), x3[:], m(5, 0, AB))        # y3
                # tails: T1 -> w' 0:32, T2 -> 32:64, T3 -> 64:96
                nc.vector.tensor_scalar_add(v[:, 8, 0:NT], P[:, XL:XH], cbs)
                e1 = st("e1", NT)
                nc.vector.tensor_add(e1[:], d1[:, XL:XH], m(5, XL, XH))
                nc.vector.tensor_scalar_add(v[:, 9, 0:NT], e1[:], cbs)    # y9 T1
                nc.vector.tensor_add(v[:, 8, NT:2 * NT], m(0, XH, YH),
                                     t0[:, XL:XH])                        # y8 T2
                nc.vector.tensor_add(v[:, 9, NT:2 * NT], v2[:], m(5, XH, YH))
                nc.gpsimd.tensor_add(v[:, 8, 2 * NT:], s1[:, XH:YH],
                                     t0[:, XH:YH])                        # y8 T3
                nc.gpsimd.tensor_add(v[:, 9, 2 * NT:], w1[:, XH:YH],
                                     h1[:, XH:YH])                        # y9 T3

            def gate_unit(g, eck, last=False):
                for c in range(2):
                    ps2 = gpsum.tile([128, 480], mybir.dt.float32, tag="gps")
                    for dck in range(NCH):
                        nc.tensor.matmul(
                            ps2[:],
                            gwr_sb[eck][:, dck * 128:(dck + 1) * 128],
                            cnn_t[g][dck][:, c * 480:(c + 1) * 480],
                            start=(dck == 0),
                            stop=(dck == NCH - 1),
                        )
                    gt = work.tile([128, 480], mybir.dt.bfloat16, tag="gate")
                    ot = work.tile([128, 480], mybir.dt.float32, tag="out")
                    chunks = ((0, 240), (240, 480)) if (last and c == 1) \
                        else ((0, 480),)
                    for lo, hi in chunks:
                        nc.scalar.activation(gt[:, lo:hi], ps2[:, lo:hi],
                                             AF.Sigmoid,
                                             bias=gb_sb[:, eck:eck + 1])
                        nc.vector.tensor_mul(ot[:, lo:hi],
                                             cnn_t[g][eck][:, c * 480 + lo:
                                                           c * 480 + hi],
                                             gt[:, lo:hi])
                        nc.sync.dma_start(
                            outT[eck * 128:(eck + 1) * 128,
                                 g * GN + c * 480 + lo:g * GN + c * 480 + hi],
                            ot[:, lo:hi])

            # phase g0 convs (cwr streams behind; xt g0 in half-chunks)
            for dck in range(NCH):
                conv_unit(0, dck)
            # g0 gates; xt g1 load issued after the first gate's outputs
            gate_unit(0, 0)
            load_xt_half(1, 0)
            load_xt_half(1, 1)
            for eck in range(1, NCH):
                gate_unit(0, eck)
            # phase g1 convs + gates
            for dck in range(NCH):
                conv_unit(1, dck)
            for eck in range(NCH):
                gate_unit(1, eck, last=(eck == NCH - 1))
    nc.compile()
    return nc


def _stream_weights(conv_w):
    W0, W1, W2 = [conv_w[:, :, k].astype(np.float64) for k in range(3)]
    g = []
    for b in PTS:
        g.append(W2 if b is INF else W0 + b * W1 + b * b * W2)
    return np.stack(g)  # [6, Dout, Din]


def _prep_weights(conv_w, conv_b, gate_w, gate_b):
    garr = _stream_weights(conv_w)[SJ]                   # [6 storage, Do, Di]
    # cwr[dck][cc, (s*8+ck)*128+dd] = g_{SJ[s]}[dck*128+dd, ck*128+cc]
    gv = garr.reshape(NS, NCH, 128, NCH, 128)            # [s, dck, dd, ck, cc]
    cw_host = np.ascontiguousarray(gv.transpose(1, 4, 0, 3, 2)).reshape(
        NCH, 128, NS * NCH * 128).astype(BF16)
    gwt = gate_w.T.reshape(NCH, 128, NCH, 128)           # [dck, dd, eck, ee]
    gw_host = np.ascontiguousarray(gwt.transpose(2, 1, 0, 3)).reshape(
        NCH, 128, NCH * 128).astype(BF16)
    cb_host = np.ascontiguousarray(conv_b.reshape(NCH, 128).T).astype(np.float32)
    gb_host = np.ascontiguousarray(gate_b.reshape(NCH, 128).T).astype(np.float32)
    return cw_host, gw_host, cb_host, gb_host


def _prep_core_x(x_shard):
    # x_shard [BC, T, D] -> xt [NG*2, 128, 3*NCH*SCOL] (storage-order halves)
    xw = x_shard.reshape(NWIN, W, D).astype(np.float64)
    xp = np.pad(xw, ((0, 0), (1, 1), (0, 0)))            # [192, 12, D]
    xt_host = np.empty((NG * 2, 128, 3 * NCH * SCOL), BF16)
    # class-major window order within a group: w' = cls*32 + w//3
    perm = np.array([3 * wi + cls for cls in range(3) for wi in range(NT)])
    for g in range(NG):
        ws = xp[g * GWN:(g + 1) * GWN]
        xA = np.einsum('ji,wic->jwc', BA, ws[perm][:, 0:6])  # [6, 96, D]
        xB = np.einsum('ji,wic->jwc', BA, ws[perm][:, 4:10])
        xTl = []
        for ci, (pts, mp) in enumerate(TAIL):
            wc = ws[ci::3][:, 8:12]                      # [32, 4, D]
            xTl.append(np.einsum('ji,wic->jwc', BT[ci], wc))
        for s in range(NS):
            j = SJ[s]
            cX, cY = XY[j]
            jX = TAIL[cX][1].index(j)
            jY = TAIL[cY][1].index(j)
            S = np.concatenate([xA[j], xB[j], xTl[cX][jX], xTl[cY][jY]],
                               axis=0)                   # [256, D]
            blk = S.T.reshape(NCH, 128, SCOL).transpose(1, 0, 2)
            h, sl = s // 3, s % 3
            xt_host[g * 2 + h, :, sl * NCH * SCOL:(sl + 1) * NCH * SCOL] = \
                blk.reshape(128, NCH * SCOL).astype(BF16)
    return xt_host


def _unshard_core(o):
    # o: [D, RC] cols ordered (g, t, cls, widx); window w = 3*widx + cls
    return np.ascontiguousarray(
        o.reshape(D, NG, W, 3, NT).transpose(1, 4, 3, 2, 0).reshape(BC, T, D))


_NC_CACHE = None


def _prep_in_maps(x, conv_w, conv_b, gate_w, gate_b):
    cw_host, gw_host, cb_host, gb_host = _prep_weights(
        conv_w, conv_b, gate_w, gate_b)
    return [
        {"xt": _prep_core_x(x[BC * i:BC * (i + 1)]), "cwr": cw_host,
         "gwr": gw_host, "cb": cb_host, "gb": gb_host}
        for i in range(NCORES)
    ]


def kernel(x, conv_w, conv_b, gate_w, gate_b):
    global _NC_CACHE
    x = np.asarray(x, np.float32)
    conv_w = np.asarray(conv_w, np.float32)
    conv_b = np.asarray(conv_b, np.float32)
    gate_w = np.asarray(gate_w, np.float32)
    gate_b = np.asarray(gate_b, np.float32)

    in_maps = _prep_in_maps(x, conv_w, conv_b, gate_w, gate_b)
    if _NC_CACHE is None:
        _NC_CACHE = _build()
    res = run_bass_kernel_spmd(_NC_CACHE, in_maps,
                               core_ids=list(range(NCORES))).results

    out = np.empty((B, T, D), np.float32)
    for i in range(NCORES):
        out[BC * i:BC * (i + 1)] = _unshard_core(np.asarray(res[i]["outT"]))
    return out
